# revision 26
# baseline (speedup 1.0000x reference)
"""Trainium2 Bass kernel for an LSTM greedy decoder (nn_Decoder).

Strategy (8 NeuronCores, SPMD):
  - vocab-shard the output projection (4000 vocab rows / core) and the
    argmax exchange; H-shard the LSTM gate computation (128 units / core).
  - per decode step: logits matmul with chunked top-8 tracking on DVE
    overlapped with the matmul; AllGather of per-core argmax candidates;
    global winner via (value, index) select with first-index tie-break;
    indirect-DMA embedding row gather; PE transpose; gates matmul; LSTM
    cell elementwise; AllGather of the new h slice.
  - matmuls run in fp32r (fp22 mantissa, 4x faster than true fp32 on the
    PE).  That injects ~1e-5 noise into logits, so the device also emits
    per-step (top1, top2, argmax); the host flags any decision whose
    top1-top2 gap is under a safety margin and replays those batch rows
    exactly in fp32 on host (rows are independent, replay is one batched
    gemm sweep), patching rows whose greedy token actually differs.
  - b_out is folded into the matmul as a K=1 ones-row; bf16 written
    logits (well within tolerance, halves device->host bytes); the
    embedding table is uploaded sharded and AllGather-ed once on device;
    donated output buffers are created on-device.
"""

import os
import sys
import numpy as np

sys.path.insert(0, "/opt/trn_rl_repo")

B = 128          # batch
H = 1024         # hidden
E = 512          # embed
V = 32000        # vocab
NC = 8           # cores
VS = V // NC     # vocab shard = 4000
HS = H // NC     # hidden units per core = 128
GS = 4 * HS      # gate rows per core = 512
NCH = 8          # logits chunks per step
CW = VS // NCH   # chunk width = 500
KH = H // 128    # h K-tiles = 8
KE = E // 128    # x K-tiles = 4

MODE = os.environ.get("K_MODE", "fp32r")   # "fp32r" | "fp32"
# host re-checks any (t,b) whose top1-top2 gap is below this margin
TIE_MARGIN = 1.5e-4 if MODE == "fp32r" else 2e-6

_CACHE = {}
LAST_EXEC_NS = None
LAST_FLAGGED = 0
LAST_REPLAYED = 0


def _build(steps, mode):
    import concourse.bass as bass
    import concourse.bacc as bacc
    import concourse.tile as tile
    from concourse import mybir
    from concourse.masks import make_identity

    f32 = mybir.dt.float32
    bf16 = mybir.dt.bfloat16
    mdt = mybir.dt.float32r if mode == "fp32r" else f32
    nc = bacc.Bacc("TRN2", target_bir_lowering=False, debug=False, num_devices=NC)

    # ---------------- I/O ----------------
    WXT = nc.dram_tensor("wxT", [E, GS], mdt, kind="ExternalInput")
    WHT = nc.dram_tensor("whT", [H, GS], mdt, kind="ExternalInput")
    BG = nc.dram_tensor("bias_g", [B, GS], f32, kind="ExternalInput")
    WOT = nc.dram_tensor("woT", [H, VS], mdt, kind="ExternalInput")
    BO = nc.dram_tensor("bo", [1, VS], mdt, kind="ExternalInput")
    EMBSH = nc.dram_tensor("embsh", [VS, E], f32, kind="ExternalInput")
    X0T = nc.dram_tensor("x0T", [E, B], mdt, kind="ExternalInput")
    H0T = nc.dram_tensor("h0T", [H, B], mdt, kind="ExternalInput")
    C0 = nc.dram_tensor("c0", [B, HS], f32, kind="ExternalInput")
    BASE = nc.dram_tensor("base", [B, 1], f32, kind="ExternalInput")

    LG = nc.dram_tensor("lg", [steps, B, VS], bf16, kind="ExternalOutput")
    VER = nc.dram_tensor("ver", [steps, B, 3], f32, kind="ExternalOutput")

    with tile.TileContext(nc) as tc:
        with (
            tc.tile_pool(name="const", bufs=1) as cpool,
            tc.tile_pool(name="weights", bufs=1) as wpool,
            tc.tile_pool(name="work", bufs=2) as work,
            tc.tile_pool(name="hbuf", bufs=2) as hpool,
            tc.tile_pool(name="cands", bufs=2) as candp,
            tc.tile_pool(name="lgps", bufs=6, space="PSUM") as lgps,
            tc.tile_pool(name="gps", bufs=2, space="PSUM") as gps,
            tc.tile_pool(name="dram", bufs=2, space="DRAM") as dr,
            tc.tile_pool(name="dram1", bufs=1, space="DRAM") as dr1,
        ):
            # ------------- one-time embedding AllGather -------------
            emb_bounce = dr1.tile([VS, E], f32)
            nc.sync.dma_start(emb_bounce[:], EMBSH.ap())
            emb_full = dr1.tile([V, E], f32, addr_space="Shared")
            nc.gpsimd.collective_compute(
                "AllGather", mybir.AluOpType.bypass,
                replica_groups=[list(range(NC))],
                ins=[emb_bounce[:]], outs=[emb_full[:]],
            )

            # ------------- resident constants / weights -------------
            ident = cpool.tile([128, 128], f32)
            make_identity(nc, ident[:])
            big64 = cpool.tile([B, 64], f32)
            nc.vector.memset(big64[:], 1e9)
            ones_f = cpool.tile([1, 128], f32)
            nc.vector.memset(ones_f[:], 1.0)
            ones_r = cpool.tile([1, 128], mdt)
            nc.vector.tensor_copy(ones_r[:], ones_f[:])
            base_sb = cpool.tile([B, 1], f32)
            nc.sync.dma_start(base_sb[:], BASE.ap())
            bias_g = cpool.tile([B, GS], f32)
            nc.sync.dma_start(bias_g[:], BG.ap())
            bo_row = cpool.tile([1, VS], mdt)
            nc.sync.dma_start(bo_row[:], BO.ap())

            wxT = wpool.tile([128, KE, GS], mdt)   # x-weight K-tiles
            nc.sync.dma_start(wxT[:], WXT.ap().rearrange("(k p) g -> p k g", p=128))
            whT = wpool.tile([128, KH, GS], mdt)   # h-weight K-tiles
            nc.sync.dma_start(whT[:], WHT.ap().rearrange("(k p) g -> p k g", p=128))
            woT = wpool.tile([128, KH, VS], mdt)   # out-proj K-tiles
            nc.sync.dma_start(woT[:], WOT.ap().rearrange("(k p) v -> p k v", p=128))

            # ------------- state -------------
            h0T_sb = hpool.tile([128, KH, B], mdt, tag="hT")
            nc.sync.dma_start(h0T_sb[:], H0T.ap().rearrange("(k p) b -> p k b", p=128))
            c_prev = hpool.tile([B, HS], f32, tag="c")
            nc.sync.dma_start(c_prev[:], C0.ap())
            x0T_sb = work.tile([128, KE, B], mdt, tag="xT")
            nc.sync.dma_start(x0T_sb[:], X0T.ap().rearrange("(k p) b -> p k b", p=128))

            def lstm_cell(hT_tiles, xT_tiles, c_in, gates_h_done=None):
                """gates matmul + cell elementwise -> (hT_next, c_next)."""
                if gates_h_done is None:
                    g_ps = gps.tile([B, GS], f32, tag="g")
                    for k in range(KH):
                        nc.tensor.matmul(
                            g_ps[:], hT_tiles[:, k, :], whT[:, k, :],
                            start=(k == 0), stop=False,
                        )
                else:
                    g_ps = gates_h_done
                for j in range(KE):
                    nc.tensor.matmul(
                        g_ps[:], xT_tiles[:, j, :], wxT[:, j, :],
                        start=False, stop=(j == KE - 1),
                    )
                g_sb = work.tile([B, GS], f32, tag="gsb")
                nc.vector.tensor_add(g_sb[:], g_ps[:], bias_g[:])
                i_sb = work.tile([B, HS], f32, tag="ig")
                f_sb = work.tile([B, HS], f32, tag="fg")
                gg_sb = work.tile([B, HS], f32, tag="gg")
                o_sb = work.tile([B, HS], f32, tag="og")
                Sig = mybir.ActivationFunctionType.Sigmoid
                Tanh = mybir.ActivationFunctionType.Tanh
                nc.scalar.activation(i_sb[:], g_sb[:, 0 * HS:1 * HS], Sig)
                nc.scalar.activation(f_sb[:], g_sb[:, 1 * HS:2 * HS], Sig)
                nc.scalar.activation(gg_sb[:], g_sb[:, 2 * HS:3 * HS], Tanh)
                nc.scalar.activation(o_sb[:], g_sb[:, 3 * HS:4 * HS], Sig)
                fc = work.tile([B, HS], f32, tag="fc")
                ig = work.tile([B, HS], f32, tag="igg")
                nc.vector.tensor_mul(fc[:], f_sb[:], c_in[:])
                nc.vector.tensor_mul(ig[:], i_sb[:], gg_sb[:])
                c_next = hpool.tile([B, HS], f32, tag="c")
                nc.vector.tensor_add(c_next[:], fc[:], ig[:])
                tanh_c = work.tile([B, HS], f32, tag="thc")
                nc.scalar.activation(tanh_c[:], c_next[:], Tanh)
                h_slice = work.tile([B, HS], f32, tag="hs")
                nc.vector.tensor_mul(h_slice[:], o_sb[:], tanh_c[:])

                # transpose h_slice -> [HS, B], allgather into full hT
                tp = lgps.tile([HS, B], f32, tag="lg")
                nc.tensor.transpose(tp[:], h_slice[:], ident[:])
                hsT = work.tile([HS, B], f32, tag="hsT")
                nc.vector.tensor_copy(hsT[:], tp[:])
                h_bounce = dr.tile([HS, B], f32, tag="hbi")
                nc.sync.dma_start(h_bounce[:], hsT[:])
                h_gath = dr.tile([H, B], f32, tag="hbo", addr_space="Shared")
                nc.gpsimd.collective_compute(
                    "AllGather", mybir.AluOpType.bypass,
                    replica_groups=[list(range(NC))],
                    ins=[h_bounce[:]], outs=[h_gath[:]],
                )
                hT_next = hpool.tile([128, KH, B], mdt, tag="hT")
                nc.sync.dma_start(
                    hT_next[:],
                    h_gath[:].bitcast(mdt).rearrange("(k p) b -> p k b", p=128))
                return hT_next, c_next

            # ------------- t=1 cell from initial state -------------
            hT_cur, c_prev = lstm_cell(h0T_sb, x0T_sb, c_prev)

            # ------------- decode steps -------------
            for t in range(1, steps + 1):
                # ---- logits matmul, chunked; top8 tracking per chunk ----
                cand_v = candp.tile([B, NCH * 8], f32, tag="cv")
                cand_i = candp.tile([B, NCH * 8], f32, tag="ci")
                for n in range(NCH):
                    ch = lgps.tile([B, CW], f32, tag="lg")
                    for k in range(KH):
                        nc.tensor.matmul(
                            ch[:], hT_cur[:, k, :], woT[:, k, n * CW:(n + 1) * CW],
                            start=(k == 0), stop=False,
                        )
                    # bias via K=1 ones-row matmul
                    nc.tensor.matmul(
                        ch[:], ones_r[:1, :], bo_row[:1, n * CW:(n + 1) * CW],
                        start=False, stop=True,
                    )
                    lch = work.tile([B, CW], f32, tag="lgch")
                    nc.vector.tensor_copy(lch[:], ch[:])
                    lchb = work.tile([B, CW], bf16, tag="lchb")
                    nc.vector.tensor_copy(lchb[:], lch[:])
                    nc.sync.dma_start(LG.ap()[t - 1, :, n * CW:(n + 1) * CW], lchb[:])
                    cvs = cand_v[:, n * 8:(n + 1) * 8]
                    nc.vector.max(cvs, lch[:])
                    ciu = work.tile([B, 8], mybir.dt.uint32, tag="ciu")
                    nc.vector.max_index(ciu[:], cvs, lch[:])
                    cif = work.tile([B, 8], f32, tag="cif")
                    nc.vector.tensor_copy(cif[:], ciu[:])
                    nc.vector.tensor_scalar_add(
                        cand_i[:, n * 8:(n + 1) * 8], cif[:], float(n * CW))

                # ---- local top2 + argmax ----
                v8 = work.tile([B, 8], f32, tag="v8")
                nc.vector.max(v8[:], cand_v[:])
                mask = work.tile([B, 64], mybir.dt.uint8, tag="m64")
                nc.vector.tensor_scalar(
                    mask[:], cand_v[:], v8[:, 0:1], None,
                    op0=mybir.AluOpType.is_equal,
                )
                sel = work.tile([B, 64], f32, tag="s64")
                nc.vector.select(sel[:], mask[:], cand_i[:], big64[:])
                my = candp.tile([B, 3], f32, tag="my")  # v1, v2, global idx
                nc.vector.tensor_copy(my[:, 0:2], v8[:, 0:2])
                mi = work.tile([B, 1], f32, tag="mi")
                nc.vector.tensor_reduce(mi[:], sel[:], mybir.AxisListType.X,
                                        mybir.AluOpType.min)
                nc.vector.tensor_add(my[:, 2:3], mi[:], base_sb[:])

                # ---- allgather candidates ----
                c_bounce = dr.tile([B, 3], f32, tag="cbi")
                nc.sync.dma_start(c_bounce[:], my[:])
                c_gath = dr.tile([B * NC, 3], f32, tag="cbo", addr_space="Shared")
                nc.gpsimd.collective_compute(
                    "AllGather", mybir.AluOpType.bypass,
                    replica_groups=[list(range(NC))],
                    ins=[c_bounce[:]], outs=[c_gath[:]],
                )
                allc = candp.tile([B, NC, 3], f32, tag="allc")
                nc.sync.dma_start(allc[:], c_gath[:].rearrange("(r b) e -> b r e", b=B))

                # ---- global winner ----
                gv8 = work.tile([B, 8], f32, tag="gv8")
                nc.vector.max(gv8[:], allc[:, :, 0:2])
                gmask = work.tile([B, NC], mybir.dt.uint8, tag="gm")
                nc.vector.tensor_scalar(
                    gmask[:], allc[:, :, 0:1].opt(),
                    gv8[:, 0:1], None, op0=mybir.AluOpType.is_equal,
                )
                gsel = work.tile([B, NC], f32, tag="gs")
                nc.vector.select(
                    gsel[:], gmask[:],
                    allc[:, :, 2:3].opt(), big64[:, 0:NC])
                ver_sb = work.tile([B, 3], f32, tag="ver")
                nc.vector.tensor_copy(ver_sb[:, 0:2], gv8[:, 0:2])
                nc.vector.tensor_reduce(
                    ver_sb[:, 2:3], gsel[:], mybir.AxisListType.X, mybir.AluOpType.min
                )
                nc.sync.dma_start(VER.ap()[t - 1], ver_sb[:])

                if t == steps:
                    break

                # ---- embedding gather of the global winner ----
                gidx = work.tile([B, 1], mybir.dt.int32, tag="gi")
                nc.vector.tensor_copy(gidx[:], ver_sb[:, 2:3])
                x_sb = work.tile([B, E], f32, tag="xsb")
                nc.gpsimd.indirect_dma_start(
                    out=x_sb[:], out_offset=None, in_=emb_full[:],
                    in_offset=bass.IndirectOffsetOnAxis(ap=gidx[:, :1], axis=0),
                )
                xT = work.tile([128, KE, B], mdt, tag="xT")
                for j in range(KE):
                    xp = lgps.tile([128, B], f32, tag="lg")
                    nc.tensor.transpose(xp[:], x_sb[:, j * 128:(j + 1) * 128], ident[:])
                    nc.vector.tensor_copy(xT[:, j, :], xp[:])

                # ---- gates-h can start right after logits (same hT) ----
                g_ps = gps.tile([B, GS], f32, tag="g")
                for k in range(KH):
                    nc.tensor.matmul(
                        g_ps[:], hT_cur[:, k, :], whT[:, k, :],
                        start=(k == 0), stop=False,
                    )
                hT_cur, c_prev = lstm_cell(None, xT, c_prev, gates_h_done=g_ps)

    nc.compile()
    return nc


def _make_runner(steps, mode):
    """Compile and return a cached callable run(in_maps) -> (outs, exec_ns)."""
    import jax
    import jax.numpy as jnp
    from jax.sharding import Mesh, PartitionSpec, NamedSharding
    from jax.experimental.shard_map import shard_map
    from concourse import bass2jax, mybir

    nc = _build(steps, mode)
    bass2jax.install_neuronx_cc_hook()

    partition_name = nc.partition_id_tensor.name if nc.partition_id_tensor else None
    in_names, out_names, out_avals = [], [], []
    for alloc in nc.m.functions[0].allocations:
        if not isinstance(alloc, mybir.MemoryLocationSet):
            continue
        name = alloc.memorylocations[0].name
        if alloc.kind == "ExternalInput":
            if name != partition_name:
                in_names.append(name)
        elif alloc.kind == "ExternalOutput":
            out_names.append(name)
            out_avals.append(jax.core.ShapedArray(
                tuple(alloc.tensor_shape), mybir.dt.np(alloc.dtype)))
    n_params = len(in_names)
    n_outs = len(out_avals)
    all_in_names = list(in_names) + list(out_names)
    if partition_name is not None:
        all_in_names.append(partition_name)

    donate = tuple(range(n_params, n_params + n_outs))

    def _body(*args):
        operands = list(args)
        if partition_name is not None:
            operands.append(bass2jax.partition_id_tensor())
        outs = bass2jax._bass_exec_p.bind(
            *operands,
            out_avals=tuple(out_avals),
            in_names=tuple(all_in_names),
            out_names=tuple(out_names),
            lowering_input_output_aliases=(),
            sim_require_finite=True,
            sim_require_nnan=True,
            nc=nc,
        )
        return tuple(outs)

    devices = jax.devices()[:NC]
    mesh = Mesh(np.asarray(devices), ("core",))
    in_specs = (PartitionSpec("core"),) * (n_params + n_outs)
    out_specs = (PartitionSpec("core"),) * n_outs
    sharded = jax.jit(
        shard_map(_body, mesh=mesh, in_specs=in_specs, out_specs=out_specs,
                  check_rep=False),
        donate_argnums=donate, keep_unused=True,
    )
    shard_ns = NamedSharding(mesh, PartitionSpec("core"))

    zero_shapes = [(NC * a.shape[0], *a.shape[1:]) for a in out_avals]
    zero_dtypes = [a.dtype for a in out_avals]
    mk_zeros = jax.jit(
        lambda: tuple(jnp.zeros(s, d) for s, d in zip(zero_shapes, zero_dtypes)),
        out_shardings=tuple(shard_ns for _ in zero_shapes),
    )

    def run(in_maps):
        import time
        from concurrent.futures import ThreadPoolExecutor
        _dbg = os.environ.get("K_TIMING")
        _t0 = time.time()

        # one upload thread per device; each streams its core's pieces
        # (a single sharded device_put serializes and is ~10x slower here)
        def _put_core(c):
            out = {}
            for name in in_names:
                a = jax.device_put(np.asarray(in_maps[c][name]), devices[c])
                a.block_until_ready()
                out[name] = a
            return out

        with ThreadPoolExecutor(max_workers=NC) as ex:
            per_core = list(ex.map(_put_core, range(NC)))
        pieces = {(name, c): per_core[c][name]
                  for c in range(NC) for name in in_names}
        dev_in = []
        for name in in_names:
            parts = [pieces[(name, c)] for c in range(NC)]
            gshape = (NC * parts[0].shape[0],) + tuple(parts[0].shape[1:])
            dev_in.append(jax.make_array_from_single_device_arrays(
                gshape, shard_ns, parts))
        for a in dev_in:
            a.block_until_ready()
        if _dbg:
            print(f"[k] upload {time.time()-_t0:.1f}s", flush=True)
        _t0 = time.time()
        zeros = mk_zeros()
        for z in zeros:
            z.block_until_ready()
        if _dbg:
            print(f"[k] zeros {time.time()-_t0:.1f}s", flush=True)
        t0 = time.time()
        out_arrs = sharded(*dev_in, *zeros)
        for a in out_arrs:
            a.block_until_ready()
        exec_ns = int((time.time() - t0) * 1e9)
        if _dbg:
            print(f"[k] exec {exec_ns/1e6:.0f}ms", flush=True)
        return {n: a for n, a in zip(out_names, out_arrs)}, exec_ns

    return run, out_names


def _get_runner(steps):
    key = (steps, MODE)
    if key not in _CACHE:
        _CACHE[key] = _make_runner(steps, MODE)
    return _CACHE[key]


def _prep_inputs(encoder_h, encoder_c, embedding, w_ih, w_hh, b_ih, b_hh,
                 w_out, b_out, sos_id):
    bias = (b_ih + b_hh).astype(np.float32)
    x0 = embedding[sos_id].astype(np.float32)            # [E]
    x0T = np.ascontiguousarray(np.broadcast_to(x0[:, None], (E, B)))
    h0T = np.ascontiguousarray(encoder_h.T)              # [H, B]
    in_maps = []
    for k in range(NC):
        rows = np.concatenate([
            np.arange(k * HS, (k + 1) * HS) + g * H for g in range(4)
        ])  # i,f,g,o rows for this core's units
        in_maps.append({
            "wxT": np.ascontiguousarray(w_ih[rows].T),
            "whT": np.ascontiguousarray(w_hh[rows].T),
            "bias_g": np.ascontiguousarray(
                np.broadcast_to(bias[rows][None, :], (B, GS))),
            "woT": np.ascontiguousarray(w_out[k * VS:(k + 1) * VS].T),
            "bo": np.ascontiguousarray(b_out[k * VS:(k + 1) * VS][None, :]),
            "embsh": np.ascontiguousarray(embedding[k * VS:(k + 1) * VS]),
            "x0T": x0T, "h0T": h0T,
            "c0": np.ascontiguousarray(encoder_c[:, k * HS:(k + 1) * HS]),
            "base": np.full((B, 1), float(k * VS), np.float32),
        })
    return in_maps


def _host_verify_and_repair(ver, inputs, steps):
    """Flag near-ties; replay the affected batch rows exactly on host
    (batched over rows); return {row_b: [steps, V] exact fp32 logits}."""
    global LAST_FLAGGED, LAST_REPLAYED

    gap = ver[:, :, 0] - ver[:, :, 1]
    flagged = np.argwhere(gap < TIE_MARGIN)
    LAST_FLAGGED = len(flagged)
    LAST_REPLAYED = 0
    if len(flagged) == 0:
        return {}

    rows = sorted({int(b) for _, b in flagged})
    R = len(rows)
    LAST_REPLAYED = R
    patches = {b: np.empty((steps, V), np.float32) for b in rows}

    w_outT = np.ascontiguousarray(inputs["w_out"].T)
    w_ihT = np.ascontiguousarray(inputs["w_ih"].T)
    w_hhT = np.ascontiguousarray(inputs["w_hh"].T)
    b_out = inputs["b_out"]
    bias = (inputs["b_ih"] + inputs["b_hh"]).astype(np.float32)
    embedding = inputs["embedding"]
    sos_id = inputs["sos_id"]

    def sigmoid(v):
        # numerically-stable fp32 sigmoid (matches jax.nn.sigmoid to ~1 ulp)
        out = np.empty_like(v)
        pos = v >= 0
        out[pos] = 1.0 / (1.0 + np.exp(-v[pos]))
        ev = np.exp(v[~pos])
        out[~pos] = ev / (1.0 + ev)
        return out

    h = inputs["encoder_h"][rows].copy()                 # [R, H]
    c = inputs["encoder_c"][rows].copy()                 # [R, H]
    x = np.broadcast_to(embedding[sos_id], (R, E)).copy()

    for t in range(1, steps + 1):
        gates = x @ w_ihT + h @ w_hhT + bias
        i_g, f_g, g_g, o_g = np.split(gates, 4, axis=-1)
        i_g = sigmoid(i_g); f_g = sigmoid(f_g)
        g_g = np.tanh(g_g); o_g = sigmoid(o_g)
        c = f_g * c + i_g * g_g
        h = o_g * np.tanh(c)
        logits = h @ w_outT + b_out                      # [R, V] exact fp32
        toks = np.argmax(logits, axis=1)
        for r, b in enumerate(rows):
            patches[b][t - 1] = logits[r]
        if t < steps:
            x = embedding[toks].copy()
    return patches


def kernel(**inputs):
    global LAST_EXEC_NS
    encoder_h = np.asarray(inputs["encoder_h"], np.float32)
    encoder_c = np.asarray(inputs["encoder_c"], np.float32)
    embedding = np.asarray(inputs["embedding"], np.float32)
    w_ih = np.asarray(inputs["w_ih"], np.float32)
    w_hh = np.asarray(inputs["w_hh"], np.float32)
    b_ih = np.asarray(inputs["b_ih"], np.float32)
    b_hh = np.asarray(inputs["b_hh"], np.float32)
    w_out = np.asarray(inputs["w_out"], np.float32)
    b_out = np.asarray(inputs["b_out"], np.float32)
    sos_id = int(np.asarray(inputs["sos_id"]))
    max_len = int(np.asarray(inputs["max_len"]))

    assert encoder_h.shape == (B, H) and w_out.shape == (V, H), "unexpected shapes"
    steps = max_len - 1

    in_maps = _prep_inputs(encoder_h, encoder_c, embedding, w_ih, w_hh,
                           b_ih, b_hh, w_out, b_out, sos_id)
    outs = None
    last_err = None
    for attempt in range(4):
        try:
            run, out_names = _get_runner(steps)
            outs, LAST_EXEC_NS = run(in_maps)
            break
        except Exception as e:  # wedged device / desynced mesh: reset + retry
            last_err = e
            _CACHE.clear()
            import time as _time
            import jax as _jax
            try:
                _jax.clear_caches()
            except Exception:
                pass
            try:
                _jax._src.api.clear_backends()
            except Exception:
                pass
            _time.sleep(5 * (attempt + 1))
    if outs is None:
        raise last_err

    from concurrent.futures import ThreadPoolExecutor
    import time as _time
    _dbg = os.environ.get("K_TIMING")
    _t0 = _time.time()

    lg = outs["lg"]
    ver_g = outs["ver"]
    full = np.empty((max_len, B, V), np.float32)
    full[0] = 0.0

    ver = np.asarray(ver_g).reshape(NC, steps, B, 3)[0]
    if _dbg:
        print(f"[k] ver fetch {_time.time()-_t0:.1f}s", flush=True)
    _t0 = _time.time()

    np_inputs = dict(w_out=w_out, b_out=b_out, w_ih=w_ih, w_hh=w_hh,
                     b_ih=b_ih, b_hh=b_hh, embedding=embedding,
                     encoder_h=encoder_h, encoder_c=encoder_c, sos_id=sos_id)

    shards = sorted(lg.addressable_shards, key=lambda s: s.index[0].start or 0)

    def _fetch(args):
        k, sh = args
        # fetch this core's [steps, B, VS] bf16 block; convert+place in-thread
        full[1:, :, k * VS:(k + 1) * VS] = np.asarray(sh.data)

    with ThreadPoolExecutor(max_workers=9) as ex:
        repair_fut = ex.submit(_host_verify_and_repair, ver, np_inputs, steps)
        list(ex.map(_fetch, enumerate(shards)))
        patches = repair_fut.result()
    if _dbg:
        print(f"[k] lg fetch+assemble+repair {_time.time()-_t0:.1f}s", flush=True)
    for b, vals in patches.items():
        full[1:, b, :] = vals
    return full
